# revision 15
# baseline (speedup 1.0000x reference)
"""ChebNetConv (K=4) distributed Bass kernel for 8 Trainium2 NeuronCores.

Strategy (graph/data parallel, pull-mode SpMM):
  - Nodes are permuted into 8x6656 padded "slots" by a degree-balanced packer.
    Core c owns output slots [6656c, 6656(c+1)). Each window of 32 slots
    receives <=256 edges from even-parity source slots and <=256 from odd
    (2+2 chunks of 128 edge-positions).
  - SpMM: per chunk, psum[96f, 32r] += G[128e, 96f].T @ S[128e, 32r] on the
    TensorEngine, where G holds per-edge source rows and S the Laplacian
    values (one column per dest row in the window).
  - Step 1 (T1 = L x): G tiles are PRE-GATHERED ON THE HOST from the input x
    (pure index staging, no FLOPs) and streamed in with plain HWDGE DMAs --
    no GpSimd descriptor generation, sequential HBM reads.
  - Steps 2,3: G tiles dma_gather'ed (f16, 256B rows) from parity-split
    DRAM tensors holding 2*T_{k-1}; AllGather (per parity) shares T_{k-1}.
    Each step is split into an even-source phase and an odd-source phase so
    the odd AllGather hides behind the even phase's gathers/matmuls.
  - Chebyshev recurrence T_k = (2 L T_{k-1}) - T_{k-2} combined from the two
    phase psums; final out.T = sum_k W_k_fm.T @ T_k_fm + b on-chip.
"""
import numpy as np

import concourse.bass as bass
import concourse.bacc as bacc
import concourse.mybir as mybir
import concourse.tile as tile
from concourse.bass_utils import run_bass_kernel_spmd

f16 = np.float16

N_CORES = 8
ROWS_PER_CORE = 6656
WIN_ROWS = 32
HALF_CAP = 256
WINS_PER_CORE = ROWS_PER_CORE // WIN_ROWS       # 208
GROUPS_PER_CORE = ROWS_PER_CORE // 128          # 52
GPAIRS = GROUPS_PER_CORE // 2                   # 26
CHUNKS_PER_CORE = WINS_PER_CORE * 4             # 832
NPAD = N_CORES * ROWS_PER_CORE                  # 53248
HALF_ROWS = NPAD // 2                           # 26624
LOC_HALF = ROWS_PER_CORE // 2                   # 3328
IN_F, OUT_F, K = 96, 128, 4
EF = 128                                        # padded row elements (f16, 256B)
NIDX = 2048                                     # indices per dma_gather call
CALLS = GPAIRS * 2                              # 52 g-tile loads/gathers per step

_compiled = None


# --------------------------------------------------------------------------
# host-side packing
# --------------------------------------------------------------------------

def _pack_rows(lap_rows, lap_cols, n):
    last = None
    for seed in range(8):
        try:
            return _pack_rows_seed(lap_rows, lap_cols, n, seed)
        except RuntimeError as e:
            last = e
    raise last


def _pack_rows_seed(lap_rows, lap_cols, n, seed=0):
    rng = np.random.default_rng(seed)
    label = np.zeros(n, np.int8)
    label[rng.permutation(n)[n // 2:]] = 1
    deg_a = np.bincount(lap_rows[label[lap_cols] == 0], minlength=n).astype(np.int64)
    deg_b = np.bincount(lap_rows[label[lap_cols] == 1], minlength=n).astype(np.int64)
    order = np.argsort(-(deg_a + deg_b), kind="stable")
    n_wins = N_CORES * WINS_PER_CORE
    wa = np.zeros(n_wins, np.int64)
    wb = np.zeros(n_wins, np.int64)
    wre = np.zeros(n_wins, np.int64)
    wro = np.zeros(n_wins, np.int64)
    row_slot = np.full(n, -1, np.int64)
    HR = WIN_ROWS // 2
    for r in order:
        a, b = deg_a[r], deg_b[r]
        lab = label[r]
        wrp = wro if lab else wre
        feas = (wrp < HR) & (wa + a <= HALF_CAP) & (wb + b <= HALF_CAP)
        if not feas.any():
            raise RuntimeError("window packing failed; graph too skewed")
        load = np.maximum(np.maximum((wa + a) / HALF_CAP, (wb + b) / HALF_CAP),
                          np.maximum((wre + (1 - lab)) / HR, (wro + lab) / HR))
        load[~feas] = 1e9
        w = int(np.argmin(load))
        pos = wro[w] * 2 + 1 if lab else wre[w] * 2
        row_slot[r] = w * WIN_ROWS + pos
        if lab:
            wro[w] += 1
        else:
            wre[w] += 1
        wa[w] += a
        wb[w] += b
    return row_slot


def _build_chunks(row_slot, lap_rows, lap_cols, lap_vals):
    """cols_half [NCHUNKS,128] int16 (source index within parity half),
    S [NCHUNKS,128,32] f32. chunk_id = gwin*16 + half*8 + (w%4)*2 + cin."""
    e_slot = row_slot[lap_rows]
    e_src = row_slot[lap_cols]
    e_half = (e_src % 2).astype(np.int64)
    e_win = e_slot // WIN_ROWS
    e_wr = e_slot % WIN_ROWS
    order = np.lexsort((e_half, e_win))
    ew, eh, ewr, esrc, ev = (e_win[order], e_half[order], e_wr[order],
                             e_src[order], lap_vals[order])
    n_wins = N_CORES * WINS_PER_CORE
    key = ew * 2 + eh
    start = np.searchsorted(key, np.arange(n_wins * 2))
    pos = np.arange(len(ew)) - start[key]
    assert pos.max() < HALF_CAP
    chunk = (ew // 4) * 16 + eh * 8 + (ew % 4) * 2 + pos // 128
    pin = pos % 128
    nchunks = n_wins * 4
    cols_half = np.zeros((nchunks, 128), np.int16)
    S = np.zeros((nchunks, 128, WIN_ROWS), np.float32)
    cols_half[chunk, pin] = (esrc // 2).astype(np.int16)
    S[chunk, pin, ewr] = ev
    return cols_half, S


def _call_cids(gp, half):
    """The 16 chunk ids covered by gather/load call (gp, half), in slot order."""
    return [(2 * gp + j // 8) * 16 + half * 8 + (j % 8) for j in range(16)]


def _idx_tile_per_core(cols_half_core):
    """[128, GROUPS_PER_CORE*128] int16 gather-index tile for one core.
    Call order: gp -> [A-call | B-call]; within a call, flat index i covers
    chunk j=i//128 (j//8 selects group 2gp+j//8, j%8 the chunk) pos i%128;
    wrapped at [i%16 replicated, call*128 + i//16]."""
    out = np.zeros((128, GROUPS_PER_CORE * 128), np.int16)
    for gp in range(GPAIRS):
        for half in (0, 1):
            call = gp * 2 + half
            flat = np.empty(NIDX, np.int16)
            for j, cid in enumerate(_call_cids(gp, half)):
                flat[j * 128:(j + 1) * 128] = cols_half_core[cid]
            blk = flat.reshape(128, 16).T               # [16, 128]
            out[:, call * 128:(call + 1) * 128] = np.tile(blk, (8, 1))
    return out


def _g1_per_core(cols_half_core, x_pad16):
    """Pre-gathered step-1 G tiles: [128, CALLS*16*IN_F] f16.
    g1[p, call, j, :] = x_pad[global_src(chunk cid(call,j), pos p)]."""
    srcs = np.empty((CALLS, 16, 128), np.int64)
    for gp in range(GPAIRS):
        for half in (0, 1):
            call = gp * 2 + half
            cids = _call_cids(gp, half)
            srcs[call] = cols_half_core[cids].astype(np.int64) * 2 + half
    g = x_pad16[srcs]                                   # [CALLS, 16, 128, IN_F]
    g = np.ascontiguousarray(g.transpose(2, 0, 1, 3))   # [128, CALLS, 16, IN_F]
    return g.reshape(128, CALLS * 16 * IN_F)


# --------------------------------------------------------------------------
# device graph
# --------------------------------------------------------------------------

def _build_nc():
    md = mybir.dt
    nc = bacc.Bacc(None, num_devices=N_CORES, num_swdge_queues=4,
                   dynamic_dma_scratch_size=32768)

    g1 = nc.declare_dram_parameter("g1", [128, CALLS * 16 * IN_F], md.float16, isOutput=False)
    x0fm = nc.declare_dram_parameter("x0fm", [IN_F, ROWS_PER_CORE], md.float16, isOutput=False)
    scoef = nc.declare_dram_parameter("scoef", [128, CHUNKS_PER_CORE * WIN_ROWS], md.float16, isOutput=False)
    idx = nc.declare_dram_parameter("idx", [128, GROUPS_PER_CORE * 128], md.int16, isOutput=False)
    wfm = nc.declare_dram_parameter("wfm", [IN_F, K * OUT_F], md.float16, isOutput=False)
    bvec = nc.declare_dram_parameter("bvec", [OUT_F, 1], md.float32, isOutput=False)
    ident = nc.declare_dram_parameter("ident", [IN_F, IN_F], md.float16, isOutput=False)
    out = nc.declare_dram_parameter("out", [OUT_F, ROWS_PER_CORE], md.float32, isOutput=True)

    rg = [list(range(N_CORES))]

    with tile.TileContext(nc) as tc:
        with (
            tc.tile_pool(name="const", bufs=1) as cp,
            tc.tile_pool(name="g1pool", bufs=5) as g1pool,
            tc.tile_pool(name="gpool", bufs=8) as gpool,
            tc.tile_pool(name="ep", bufs=3) as ep,
            tc.tile_pool(name="ps_spmm", bufs=4, space="PSUM") as ps_spmm,
            tc.tile_pool(name="ps_t", bufs=2, space="PSUM") as ps_t,
            tc.tile_pool(name="ps_o", bufs=2, space="PSUM") as ps_o,
            tc.tile_pool(name="dram", bufs=1, space="DRAM") as dp,
        ):
            scoef_sb = cp.tile([128, CHUNKS_PER_CORE * WIN_ROWS], md.float16)
            idx_sb = cp.tile([128, GROUPS_PER_CORE * 128], md.int16)
            t0fm = cp.tile([IN_F, ROWS_PER_CORE], md.float16)
            t1fm = cp.tile([IN_F, ROWS_PER_CORE], md.float16)
            t2fm = cp.tile([IN_F, ROWS_PER_CORE], md.float16)
            t3fm = cp.tile([IN_F, ROWS_PER_CORE], md.float16)
            wfm_sb = cp.tile([IN_F, K * OUT_F], md.float16)
            b_sb = cp.tile([OUT_F, 1], md.float32)
            ident_sb = cp.tile([IN_F, IN_F], md.float16)

            # sync queue: only what the first step-1 gpairs need, then g1
            # streams; everything else on scalar (idle until first epilogue).
            QS = CHUNKS_PER_CORE * WIN_ROWS // 4
            nc.sync.dma_start(out=scoef_sb[:, 0:QS], in_=scoef[:, 0:QS])
            nc.scalar.dma_start(out=ident_sb[:], in_=ident[:])
            for q in range(1, 4):
                nc.scalar.dma_start(out=scoef_sb[:, q * QS:(q + 1) * QS],
                                    in_=scoef[:, q * QS:(q + 1) * QS])
            nc.scalar.dma_start(out=t0fm[:], in_=x0fm[:])
            nc.scalar.dma_start(out=idx_sb[:, 0:3328], in_=idx[:, 0:3328])
            nc.scalar.dma_start(out=idx_sb[:, 3328:], in_=idx[:, 3328:])
            nc.scalar.dma_start(out=wfm_sb[:], in_=wfm[:])
            nc.scalar.dma_start(out=b_sb[:], in_=bvec[:])

            tfm = [t0fm, t1fm, t2fm, t3fm]
            g1r = g1[:].rearrange("p (call s f) -> p call s f", call=CALLS, s=16)

            # internal DRAM for the T_k exchange (k = 1, 2)
            tloc = {}
            tfull = {}
            for k in (1, 2):
                for h, tag in ((0, "e"), (1, "o")):
                    tloc[(k, h)] = dp.tile([LOC_HALF, EF], md.float16,
                                           name=f"t{k}{tag}loc")
                    tfull[(k, h)] = dp.tile([HALF_ROWS, EF], md.float16,
                                            addr_space="Shared", name=f"t{k}{tag}full")

            def epilogue(k, g, gsl, rm4_box):
                """2*T_k -> transposed row-major halves -> tloc DRAM (k < 3)."""
                fm2 = ep.tile([IN_F, 128], md.float16, tag="fm2")
                nc.scalar.mul(out=fm2[:], in_=tfm[k][:, gsl], mul=2.0)
                fm2r = fm2[:].rearrange("p (s two) -> p two s", two=2)
                if g % 4 == 0:
                    rm4_box[0] = [ep.tile([64, 4, IN_F], md.float16,
                                          tag="rm4e", name=f"rm4e_{k}_{g}"),
                                  ep.tile([64, 4, IN_F], md.float16,
                                          tag="rm4o", name=f"rm4o_{k}_{g}")]
                rm4 = rm4_box[0]
                for h2 in (0, 1):
                    pst = ps_t.tile([64, IN_F], md.float16, space="PSUM", tag="pst")
                    nc.tensor.transpose(
                        out=pst[:], in_=fm2r[:, h2, :],
                        identity=ident_sb[:],
                    )
                    nc.scalar.copy(out=rm4[h2][:, g % 4, :], in_=pst[:])
                    if g % 4 == 3:
                        dst = tloc[(k, h2)][:].rearrange(
                            "(Q q r) f -> Q r q f", q=4, r=64)
                        nc.scalar.dma_start(
                            out=dst[g // 4, :, :, 0:IN_F],
                            in_=rm4[h2][:],
                        )

            # ---------------- step 1: T1 = L x, G tiles streamed from DRAM
            rm4_box = [None]
            g1p = g1[:].rearrange("p (gp s f) -> p gp s f", gp=GPAIRS, s=32)
            for gp in range(GPAIRS):
                gt = g1pool.tile([128, 2, 16, IN_F], md.float16, tag="g1",
                                 name=f"g1_{gp}")
                nc.sync.dma_start(out=gt[:], in_=g1p[:, gp, :, :].rearrange(
                    "p (h s) f -> p h s f", h=2))
                for h in (0, 1):
                    g = 2 * gp + h
                    psum = ps_spmm.tile([IN_F, 128], md.float32, space="PSUM", tag="pspmm")
                    for w in range(4):
                        for half in (0, 1):
                            for c in range(2):
                                cid = g * 16 + half * 8 + w * 2 + c
                                slot = h * 8 + w * 2 + c
                                nc.tensor.matmul(
                                    out=psum[0:IN_F, w * WIN_ROWS:(w + 1) * WIN_ROWS],
                                    lhsT=gt[:, half, slot, 0:IN_F],
                                    rhs=scoef_sb[:, cid * WIN_ROWS:(cid + 1) * WIN_ROWS],
                                    start=(half == 0 and c == 0),
                                    stop=(half == 1 and c == 1),
                                )
                    gsl = slice(g * 128, (g + 1) * 128)
                    nc.scalar.copy(out=tfm[1][:, gsl], in_=psum[:])
                    epilogue(1, g, gsl, rm4_box)

            # ---------------- steps 2,3: even phase || AG(odd), then odd phase
            qn = 0
            for k in (2, 3):
                nc.gpsimd.collective_compute(
                    "AllGather", mybir.AluOpType.bypass,
                    replica_groups=rg,
                    ins=[tloc[(k - 1, 0)][:]],
                    outs=[tfull[(k - 1, 0)][:]],
                )
                src = (tfull[(k - 1, 0)][:], tfull[(k - 1, 1)][:])
                for phase in (0, 1):
                    rm4_box = [None]
                    for gp in range(GPAIRS):
                        call = gp * 2 + phase
                        g_sb = gpool.tile([128, 16, EF], md.float16, tag="g",
                                          name=f"g_{k}_{phase}_{gp}")
                        nc.gpsimd.dma_gather(
                            g_sb[:], src[phase],
                            idx_sb[:, call * 128:(call + 1) * 128],
                            NIDX, NIDX, EF,
                            single_packet=False, queue_num=qn,
                        )
                        qn = (qn + 1) % 4
                        if phase == 0 and gp == 4:
                            nc.gpsimd.collective_compute(
                                "AllGather", mybir.AluOpType.bypass,
                                replica_groups=rg,
                                ins=[tloc[(k - 1, 1)][:]],
                                outs=[tfull[(k - 1, 1)][:]],
                            )
                        for h in (0, 1):
                            g = 2 * gp + h
                            psum = ps_spmm.tile([IN_F, 128], md.float32,
                                                space="PSUM", tag="pspmm")
                            for w in range(4):
                                for c in range(2):
                                    cid = g * 16 + phase * 8 + w * 2 + c
                                    slot = h * 8 + w * 2 + c
                                    nc.tensor.matmul(
                                        out=psum[0:IN_F, w * WIN_ROWS:(w + 1) * WIN_ROWS],
                                        lhsT=g_sb[:, slot, 0:IN_F],
                                        rhs=scoef_sb[:, cid * WIN_ROWS:(cid + 1) * WIN_ROWS],
                                        start=(w == 0 and c == 0),
                                        stop=(w == 3 and c == 1),
                                    )
                            gsl = slice(g * 128, (g + 1) * 128)
                            if phase == 0:
                                nc.scalar.copy(out=tfm[k][:, gsl], in_=psum[:])
                            else:
                                tmp = ep.tile([IN_F, 128], md.float16, tag="tmp")
                                nc.vector.tensor_sub(tmp[:], psum[:],
                                                     tfm[k - 2][:, gsl])
                                nc.vector.tensor_add(tfm[k][:, gsl],
                                                     tfm[k][:, gsl], tmp[:])
                                if k < 3:
                                    epilogue(k, g, gsl, rm4_box)
                                else:
                                    po = ps_o.tile([OUT_F, 128], md.float32,
                                                   space="PSUM", tag="po")
                                    for kk in range(K):
                                        nc.tensor.matmul(
                                            out=po[:],
                                            lhsT=wfm_sb[:, kk * OUT_F:(kk + 1) * OUT_F],
                                            rhs=tfm[kk][:, gsl],
                                            start=(kk == 0),
                                            stop=(kk == K - 1),
                                        )
                                    osb = ep.tile([OUT_F, 128], md.float32, tag="osb")
                                    nc.vector.tensor_add(
                                        osb[:], po[:],
                                        b_sb[:, 0:1].to_broadcast([OUT_F, 128]))
                                    nc.sync.dma_start(out=out[:, gsl], in_=osb[:])


    nc.finalize()
    return nc


# --------------------------------------------------------------------------
# entry point
# --------------------------------------------------------------------------

def kernel(x, lap_rows, lap_cols, lap_vals, W, b):
    global _compiled
    x = np.asarray(x, np.float32)
    lap_rows = np.asarray(lap_rows, np.int32)
    lap_cols = np.asarray(lap_cols, np.int32)
    lap_vals = np.asarray(lap_vals, np.float32)
    W = np.asarray(W, np.float32)
    b = np.asarray(b, np.float32)
    n = x.shape[0]

    row_slot = _pack_rows(lap_rows, lap_cols, n)
    cols_half, S = _build_chunks(row_slot, lap_rows, lap_cols, lap_vals)

    x_pad = np.zeros((NPAD, IN_F), np.float32)
    x_pad[row_slot] = x
    x_pad16 = x_pad.astype(f16)

    Wr = W.reshape(OUT_F, IN_F, K)
    wfm = np.ascontiguousarray(
        Wr.transpose(1, 2, 0).reshape(IN_F, K * OUT_F)).astype(f16)
    bvec = b.reshape(OUT_F, 1).astype(np.float32)
    ident = np.eye(IN_F, dtype=f16)

    in_maps = []
    for c in range(N_CORES):
        csl = slice(c * CHUNKS_PER_CORE, (c + 1) * CHUNKS_PER_CORE)
        S_c = S[csl].astype(f16)                    # [832, 128, 32]
        scoef_c = np.ascontiguousarray(
            S_c.transpose(1, 0, 2).reshape(128, CHUNKS_PER_CORE * WIN_ROWS))
        idx_c = _idx_tile_per_core(cols_half[csl])
        g1_c = _g1_per_core(cols_half[csl], x_pad16)
        x0fm_c = np.ascontiguousarray(
            x_pad[c * ROWS_PER_CORE:(c + 1) * ROWS_PER_CORE].T).astype(f16)
        in_maps.append({
            "g1": g1_c, "x0fm": x0fm_c, "scoef": scoef_c,
            "idx": idx_c, "wfm": wfm, "bvec": bvec, "ident": ident,
        })

    global _last_in_maps
    _last_in_maps = in_maps
    if _compiled is None:
        _compiled = _build_nc()
    res = run_bass_kernel_spmd(_compiled, in_maps, core_ids=list(range(N_CORES)))
    out_pad = np.concatenate(
        [res.results[c]["out"] for c in range(N_CORES)], axis=1).T  # [NPAD, 128]
    return np.ascontiguousarray(out_pad[row_slot]).astype(np.float32)


if __name__ == "__main__":
    import time
    d = np.load("inputs.npz")
    t0 = time.time()
    y = kernel(**{k: d[k] for k in d.files})
    print(f"kernel {time.time()-t0:.1f}s")
    expected = np.load("expected.npy")
    rel = np.linalg.norm(y - expected) / np.linalg.norm(expected)
    print(f"rel_err {rel:.3e}")
